# revision 18
# baseline (speedup 1.0000x reference)
"""LinearAttention Trainium2 kernel v2: data-parallel over batch on 8 cores.

v2 change vs v1: the V projection is folded into the context build the same
way v1 folded the Q side into F.  v1 computed V^T = x^T Wv^T (32768 PE cyc
per batch) and then ctx = sum_l exp(k) V^T (16512 cyc).  But
  ctx_h[d,e] = sum_l exp(k[d,l]) v_h[e,l] = sum_c PT[c,d] WvT[c,e],
  PT[c,d]   = sum_l x[c,l] exp(k[d,l]),
so contracting x with exp(K)^T over L directly (lhsT = x^T chunks, rhs =
expkt, 32768 cyc) leaves only a tiny 256-contraction (1024 cyc) against the
Wv^T weights.  Net: ~49.3k -> ~34.0k PE cyc per batch on the V side, and the
16384-elem/partition V^T PSUM->SBUF copy disappears from DVE.

softmax denominator: den[d] = sum_l expkt[l,d] needs a partition-transposing
reduction.  Doing it as 128 one-column PE matmuls costs ~13us/rep of EXPOSED
weight loads (the PE has a single shadow weight buffer: each long stream can
hide only ONE upcoming 128-row load, and psk/PT already claim all four).
Instead the lp-chunks of expkt are pre-reduced elementwise on the vector
engines (even lp on DVE, odd lp on Pool, combine on DVE), leaving only 4
one-column PE matmuls per batch on the reduced [128, 512] tile.

PE work per batch: K proj 32768 + PT 32768 + den ~4 + ctx 1024 + H 1024
+ F^T 2048 + final 16384 ~= 86.1k cyc (v1: 101.5k).

PSUM banks: psk pool 2 + PT 2 + den 1 + tail 3 (ctx lives in tail) = 8.
Everything else (F build, final GEMM y = F^T.T x + b, schedules, timing
scaffolding) follows v1.
"""

import numpy as np

B, C, L = 16, 256, 4096
HID = 512
N_CORES = 8
NB = B // N_CORES  # batches per core
CC = C // 128  # contraction chunks for 256-channel GEMMs (2)
LP = L // 128  # l-tiles with l on partitions (32)
LT = L // 512  # l-tiles of 512 for moving-dim matmuls (8)
PR = HID // 128  # head-pairs (4): each 128-wide chunk = 2 heads of 64

_CACHE = {}

# timing-experiment knockouts (leave False for correctness)
_KO_ADDS = False
_KO_XT = False
_KO_TAIL = False
_KO_OUT = False


def _build(reps=1, timing=False):
    from concourse import bacc, mybir, tile
    import concourse.bass as bass

    bf16 = mybir.dt.bfloat16
    f32 = mybir.dt.float32
    Exp = mybir.ActivationFunctionType.Exp
    Copy = mybir.ActivationFunctionType.Copy
    Ident = mybir.ActivationFunctionType.Identity

    nc = bacc.Bacc(
        "TRN2",
        target_bir_lowering=False,
        debug=False,
        enable_asserts=False,
        num_devices=N_CORES,
    )

    x_d = nc.dram_tensor(
        "x", [NB, CC, 128, L], bf16, kind="Internal" if timing else "ExternalInput"
    )
    xT_d = nc.dram_tensor(
        "xT",
        [NB, 128, LP, CC, 128],
        bf16,
        kind="Internal" if timing else "ExternalInput",
    )
    wkv_d = nc.dram_tensor("wkv", [128, CC, 2, HID], bf16, kind="ExternalInput")
    wqo_d = nc.dram_tensor("wqo", [128, 2, PR, C], bf16, kind="ExternalInput")
    bb_d = nc.dram_tensor("bb", [128, 2], f32, kind="ExternalInput")
    if timing:
        nslice = min(reps, 32)
        out_d = nc.dram_tensor(
            "scratch", [nslice * NB, 2, 128, L], bf16, kind="Internal"
        )
        chk_d = nc.dram_tensor("chk", [128, reps], bf16, kind="ExternalOutput")
    else:
        out_d = nc.dram_tensor("out", [NB, 2, 128, L], bf16, kind="ExternalOutput")
        chk_d = None

    with tile.TileContext(nc) as tc:
        with (
            tc.tile_pool(name="const", bufs=1) as const,
            tc.tile_pool(name="xp", bufs=2) as xp,
            tc.tile_pool(name="small", bufs=2) as small,
            tc.tile_pool(name="ostp", bufs=3) as ostp,
            tc.tile_pool(name="ps_proj", bufs=2, space="PSUM") as ps_proj,
            tc.tile_pool(name="ps_pt", bufs=1, space="PSUM") as ps_pt,
            tc.tile_pool(name="ps_den", bufs=1, space="PSUM") as ps_den,
            tc.tile_pool(name="ps_tail", bufs=3, space="PSUM") as ps_tail,
        ):
            wkv = const.tile([128, CC, 2, HID], bf16)
            wqo = const.tile([128, 2, PR, C], bf16)
            bb = const.tile([128, 2], f32)
            ctxs = const.tile([128, PR, 128], bf16)
            expkt = const.tile([128, LP, HID], bf16)
            ones = const.tile([128, 1], bf16)

            nc.scalar.dma_start(wkv[:, 0], wkv_d[:, 0])
            nc.sync.dma_start(wkv[:, 1], wkv_d[:, 1])
            nc.scalar.dma_start(wqo[:], wqo_d[:])
            nc.scalar.dma_start(bb[:], bb_d[:])
            nc.gpsimd.memset(ones[:], 1.0)
            wqn, wo = wqo[:, 0], wqo[:, 1]

            for rep in range(reps):
                if timing:
                    chk = small.tile([128, 1], bf16, tag="chk")
                    nc.sync.dma_start(chk[:], out_d[(rep % nslice) * NB, 0, :, 0:1])
                    nc.sync.dma_start(chk_d[:, rep : rep + 1], chk[:])
                xts, xTts = [], []
                for bi in range(NB):
                    xts.append(
                        xp.tile([128, CC, L], bf16, tag="xt", name=f"xt_{rep}_{bi}")
                    )
                    xTts.append(
                        xp.tile(
                            [128, LP, CC, 128], bf16, tag="xtT", name=f"xtT_{rep}_{bi}"
                        )
                    )
                # x on the Pool (SWDGE) queue, xT on the DVE queue: both
                # decoupled from the SP/ACT output queues; first chunks small
                # so the first projections / first PT matmuls start sooner.
                xbounds = [0, 512, 1024, 2048, L]
                lpbounds = [0, 4, 8, 20, LP]
                for bi in range(NB):
                    for xc in range(len(xbounds) - 1):
                        for cc in range(CC):
                            nc.gpsimd.dma_start(
                                xts[bi][:, cc, xbounds[xc] : xbounds[xc + 1]],
                                x_d[bi, cc, :, xbounds[xc] : xbounds[xc + 1]],
                            )
                        # interleave the x^T chunk right behind the matching
                        # x chunk so the first PT matmul isn't starved behind
                        # the whole 2MB x load on the Pool queue.
                        if not _KO_XT:
                            nc.gpsimd.dma_start(
                                xTts[bi][:, lpbounds[xc] : lpbounds[xc + 1]],
                                xT_d[bi, :, lpbounds[xc] : lpbounds[xc + 1]],
                            )

                def lp_phase(bi):
                    xt, xtT = xts[bi], xTts[bi]
                    ptp = ps_pt.tile(
                        [128, CC, HID], f32, tag="pt", name=f"pt_{rep}_{bi}"
                    )
                    # expkt lp-chunk partial sums for den: even lp on DVE,
                    # odd lp on Pool (bf16 accumulate; ~0.3% den rounding,
                    # well inside budget)
                    accD = small.tile([128, HID], bf16, tag="accD")
                    accP = small.tile([128, HID], bf16, tag="accP")
                    for lp in range(LP + 1):
                        if lp < LP:
                            psk = ps_proj.tile([128, HID], f32, tag="mm")
                            for cc in range(CC):
                                nc.tensor.matmul(
                                    psk[:],
                                    xt[:, cc, lp * 128 : (lp + 1) * 128],
                                    wkv[:, cc, 0, :],
                                    start=(cc == 0),
                                    stop=(cc == CC - 1),
                                )
                            nc.scalar.activation(expkt[:, lp, :], psk[:], Exp)
                            if _KO_ADDS:
                                if lp < 2:
                                    (nc.vector if lp == 0 else nc.gpsimd).memset(
                                        (accD if lp == 0 else accP)[:], 1.0
                                    )
                            else:
                                eng = nc.vector if lp % 2 == 0 else nc.gpsimd
                                acc = accD if lp % 2 == 0 else accP
                                if lp < 2:
                                    eng.tensor_copy(acc[:], expkt[:, lp, :])
                                else:
                                    eng.tensor_add(acc[:], expkt[:, lp, :], acc[:])
                        if 0 < lp:
                            lq = lp - 1
                            for cc in range(CC):
                                nc.tensor.matmul(
                                    ptp[:, cc, :],
                                    xtT[:, lq, cc, :],
                                    expkt[:, lq, :],
                                    start=(lq == 0),
                                    stop=(lq == LP - 1),
                                )
                    if not _KO_ADDS:
                        nc.vector.tensor_add(accD[:], accP[:], accD[:])
                    dnp = ps_den.tile([128, PR], f32, tag="den", name=f"dn_{rep}_{bi}")
                    for pr in range(PR):
                        nc.tensor.matmul(
                            dnp[:, pr : pr + 1],
                            accD[:, pr * 128 : (pr + 1) * 128],
                            ones[:],
                            start=(pr == 0),
                            stop=(pr == PR - 1),
                        )
                    return ptp, dnp

                def hf_phase(bi, ptp, dnp):
                    inv_den = small.tile([128, PR], f32, tag="invden")
                    nc.vector.reciprocal(inv_den[:], dnp[:])
                    pts = small.tile([128, CC, HID], bf16, tag="pts")
                    for cc in range(CC):
                        nc.vector.tensor_copy(pts[:, cc, :], ptp[:, cc, :])
                    # ctx[d,e] = sum_c PT[c,d] WvT[c,e]; all 4 head-pair
                    # blocks packed in one PSUM bank, one start/stop chain.
                    ctxp = ps_tail.tile(
                        [128, PR, 128], f32, tag="mm", name=f"ctx_{rep}_{bi}"
                    )
                    for pr in range(PR):
                        for cc in range(CC):
                            nc.tensor.matmul(
                                ctxp[:, pr, :],
                                pts[:, cc, pr * 128 : (pr + 1) * 128],
                                wkv[:, cc, 1, pr * 128 : (pr + 1) * 128],
                                start=(pr == 0 and cc == 0),
                                stop=(pr == PR - 1 and cc == CC - 1),
                            )
                    # 1/den-scaled copies split ACT/DVE, cross-head 64x64
                    # quadrants zeroed on Pool.
                    for pr in range(PR):
                        if pr < 2:
                            nc.scalar.activation(
                                ctxs[:, pr, :],
                                ctxp[:, pr, :],
                                Copy,
                                scale=inv_den[:, pr : pr + 1],
                            )
                        else:
                            nc.vector.tensor_scalar_mul(
                                ctxs[:, pr, :],
                                ctxp[:, pr, :],
                                inv_den[:, pr : pr + 1],
                            )
                        nc.gpsimd.memset(ctxs[0:64, pr, 64:128], 0.0)
                        nc.gpsimd.memset(ctxs[64:128, pr, 0:64], 0.0)

                    hs = small.tile([128, PR, C], bf16, tag="hs")
                    for pr in range(PR):
                        hp = ps_tail.tile([128, C], f32, tag="mm")
                        nc.tensor.matmul(
                            hp[:], ctxs[:, pr, :], wqn[:, pr, :], start=True, stop=True
                        )
                        nc.vector.tensor_copy(hs[:, pr, :], hp[:])
                    fts = small.tile([128, CC, C], bf16, tag="fts")
                    for cc in range(CC):
                        ftp = ps_tail.tile([128, C], f32, tag="mm")
                        for pr in range(PR):
                            nc.tensor.matmul(
                                ftp[:],
                                hs[:, pr, cc * 128 : (cc + 1) * 128],
                                wo[:, pr, :],
                                start=(pr == 0),
                                stop=(pr == PR - 1),
                            )
                        nc.vector.tensor_copy(fts[:, cc, :], ftp[:])
                    return fts

                def fx_phase(bi, fts, lo=0, hi=LT // 2):
                    xt = xts[bi]
                    for ltp in range(lo, hi):
                        ostg = ostp.tile([128, 2, 1024], bf16, tag="ostg")
                        for lth in range(2):
                            lt = 2 * ltp + lth
                            for oc2 in range(2):
                                psf = ps_tail.tile([128, 512], f32, tag="mm")
                                for cc in range(CC):
                                    nc.tensor.matmul(
                                        psf[:],
                                        fts[:, cc, oc2 * 128 : (oc2 + 1) * 128],
                                        xt[:, cc, lt * 512 : (lt + 1) * 512],
                                        start=(cc == 0),
                                        stop=(cc == CC - 1),
                                    )
                                dst = ostg[:, oc2, lth * 512 : (lth + 1) * 512]
                                if oc2 == 0:
                                    nc.scalar.activation(
                                        dst, psf[:], Ident, bias=bb[:, oc2 : oc2 + 1]
                                    )
                                else:
                                    nc.vector.tensor_scalar_add(
                                        dst, psf[:], bb[:, oc2 : oc2 + 1]
                                    )
                        obi = ((rep % (min(reps, 32))) * NB + bi) if timing else bi
                        ocol = ltp * 1024
                        last = bi == NB - 1 and ltp == LT // 2 - 1
                        for oc2 in range(2):
                            if _KO_OUT:
                                continue
                            if last:
                                nc.sync.dma_start(
                                    out_d[obi, oc2, :, ocol : ocol + 512],
                                    ostg[:, oc2, 0:512],
                                )
                                (nc.scalar if oc2 else nc.sync).dma_start(
                                    out_d[obi, oc2, :, ocol + 512 : ocol + 1024],
                                    ostg[:, oc2, 512:1024],
                                )
                            else:
                                nc.sync.dma_start(
                                    out_d[obi, oc2, :, ocol : ocol + 1024],
                                    ostg[:, oc2, :],
                                )

                if _KO_TAIL:
                    for bi in range(NB):
                        ptp, dnp = lp_phase(bi)
                        pts = small.tile([128, CC, HID], bf16, tag="pts")
                        for cc in range(CC):
                            nc.vector.tensor_copy(pts[:, cc, :], ptp[:, cc, :])
                        invd = small.tile([128, PR], f32, tag="invden")
                        nc.vector.reciprocal(invd[:], dnp[:])
                        obi = ((rep % (min(reps, 32))) * NB + bi) if timing else bi
                        nc.sync.dma_start(
                            out_d[obi, 0, :, 0:512], pts[:, 0, 0:512]
                        )
                else:
                    prev = None
                    for bi in range(NB):
                        ptp, dnp = lp_phase(bi)
                        if bi + 1 < NB:
                            prev = (bi, hf_phase(bi, ptp, dnp))
                        else:
                            if prev is not None:
                                fx_phase(*prev, 0, 2)
                            fts = hf_phase(bi, ptp, dnp)
                            if prev is not None:
                                fx_phase(*prev, 2, LT // 2)
                            fx_phase(bi, fts)

    nc.compile()
    return nc


def _get_nc():
    if "nc" not in _CACHE:
        _CACHE["nc"] = _build()
    return _CACHE["nc"]


def _prep_in_maps(x, w_qkv, w_out, b_out):
    import ml_dtypes

    bf16 = ml_dtypes.bfloat16
    wqn = w_qkv[0:512].reshape(PR, 128, C)
    wk_t = np.ascontiguousarray(w_qkv[512:1024].T).reshape(CC, 128, HID)
    wv_t = np.ascontiguousarray(w_qkv[1024:1536].T).reshape(CC, 128, HID)
    wo_t = np.ascontiguousarray(w_out.T).reshape(PR, 128, C)
    wkv = np.ascontiguousarray(
        np.stack([wk_t, wv_t], axis=1).transpose(2, 0, 1, 3)
    ).astype(bf16)
    wqo = np.ascontiguousarray(
        np.stack([wqn.transpose(1, 0, 2), wo_t.transpose(1, 0, 2)], axis=1)
    ).astype(bf16)
    bb = np.ascontiguousarray(b_out.reshape(2, 128).T).astype(np.float32)
    in_maps = []
    for c in range(N_CORES):
        xc = x[c * NB : (c + 1) * NB]
        xs = xc.reshape(NB, CC, 128, L).astype(bf16)
        # x^T per batch: [l, c] -> [128(l in chunk), LP, CC, 128(c)]
        xT = np.ascontiguousarray(
            xc.transpose(0, 2, 1).reshape(NB, LP, 128, CC, 128).transpose(0, 2, 1, 3, 4)
        ).astype(bf16)
        in_maps.append(
            {
                "x": np.ascontiguousarray(xs),
                "xT": xT,
                "wkv": wkv,
                "wqo": wqo,
                "bb": bb,
            }
        )
    return in_maps


def kernel(x, w_qkv, w_out, b_out):
    from concourse.bass_utils import run_bass_kernel_spmd

    nc = _get_nc()
    in_maps = _prep_in_maps(
        np.asarray(x, dtype=np.float32),
        np.asarray(w_qkv, dtype=np.float32),
        np.asarray(w_out, dtype=np.float32),
        np.asarray(b_out, dtype=np.float32),
    )
    res = run_bass_kernel_spmd(nc, in_maps, core_ids=list(range(N_CORES)))
    out = np.concatenate(
        [
            res.results[c]["out"].astype(np.float32).reshape(NB, C, L)
            for c in range(N_CORES)
        ],
        axis=0,
    )
    return out
